# revision 1
# baseline (speedup 1.0000x reference)
"""Ragged-batch dual single-head attention (AttentionLayer) for Trainium2, 8 NeuronCores.

Data-parallel over graphs: 16 graphs per core, contiguous node segments
(batch_ids is sorted). Params replicated.

Algebraic restructuring (single head, one query per graph):
  energy[g, n] = (Q[g] @ kw) . x[n] / sqrt(768) + const(g)
    -> q_tilde = Q @ kw computed once per graph, K projection of nodes never
       materialized. The per-graph const (Q.kb) cancels in softmax - dropped.
  out[g] = (attn[g, :] @ X) @ vw.T @ ow.T + bias
    -> V projection never materialized; attn-weighted X context (768-dim) is
       projected afterwards. vb contributes a constant vector - folded into a
       host-precomputed output bias.

On-device layout is "node-major / hidden-major": all per-graph vectors live as
columns ([dim, graph]), energies as [node, graph], so no transposes are needed
anywhere. Compute dtype bf16 (f32 PSUM accumulate).
"""

import os
from contextlib import ExitStack

import numpy as np
import ml_dtypes

import concourse.bass as bass
import concourse.tile as tile
from concourse import bacc, mybir
from concourse.bass_utils import run_bass_kernel_spmd

BF16 = ml_dtypes.bfloat16
FP8 = ml_dtypes.float8_e4m3
WSCALE = 16.0
HID = 768
GENE = 512
B = 128
NCORES = 8
G = B // NCORES  # graphs per core
T = 512          # nodes per supertile
SCALE = 1.0 / float(np.sqrt(HID))

_BUILD_CACHE = {}


def _build(C, dbg=False, num_devices=NCORES):
    """Build + compile the SPMD Bass graph for per-core node capacity C."""
    ns = C // T
    dt = mybir.dt
    BF = dt.bfloat16
    F32 = dt.float32
    F8 = dt.float8e4

    nc = bacc.Bacc("TRN2", target_bir_lowering=False, debug=False, num_devices=num_devices)
    dbg_e = {}
    if dbg:
        dbg_e["qt_all"] = nc.declare_dram_parameter("dbg_qt", [128, 6, 2 * G], dt.float32, isOutput=True)
        dbg_e["pt0"] = nc.declare_dram_parameter("dbg_pt0", [128, 4, 2 * G], dt.float32, isOutput=True)
        dbg_e["ctx"] = nc.declare_dram_parameter("dbg_ctx", [128, 6, 2 * G], dt.float32, isOutput=True)
        dbg_e["l"] = nc.declare_dram_parameter("dbg_l", [1, 4, 2 * G], dt.float32, isOutput=True)
        dbg_e["ctxT"] = nc.declare_dram_parameter("dbg_ctxT", [128, 6, 2 * G], dt.float32, isOutput=True)

    xt_e = nc.declare_dram_parameter("xt4", [ns, 128, 6, T], BF, isOutput=False)
    xn_e = nc.declare_dram_parameter("xn", [C, HID], BF, isOutput=False)
    mk_e = nc.declare_dram_parameter("mask4", [ns, 128, 4, 2 * G], BF, isOutput=False)
    gin_e = [
        nc.declare_dram_parameter("geneT", [640, G], BF, isOutput=False),
        nc.declare_dram_parameter("bionT", [640, G], BF, isOutput=False),
    ]
    fc_e = [nc.declare_dram_parameter(f"fc{a}T", [640, HID], BF, isOutput=False) for a in range(2)]
    qw_e = [nc.declare_dram_parameter(f"qw{a}T", [896, HID], BF, isOutput=False) for a in range(2)]
    kw_e = [nc.declare_dram_parameter(f"kw{a}", [HID, HID], BF, isOutput=False) for a in range(2)]
    vw_e = [nc.declare_dram_parameter(f"vw{a}T", [HID, HID], BF, isOutput=False) for a in range(2)]
    ow_e = [nc.declare_dram_parameter(f"ow{a}T", [HID, HID], BF, isOutput=False) for a in range(2)]
    ob_e = nc.declare_dram_parameter("out_bias_pb", [128, 6], F32, isOutput=False)
    out_e = nc.declare_dram_parameter("out", [HID, G], F32, isOutput=True)

    with tile.TileContext(nc) as tc, ExitStack() as ctx:
        wpool = ctx.enter_context(tc.tile_pool(name="weights", bufs=1))
        apool = ctx.enter_context(tc.tile_pool(name="phasea", bufs=1))
        xtp = ctx.enter_context(tc.tile_pool(name="xt", bufs=6))
        xnp = ctx.enter_context(tc.tile_pool(name="xn", bufs=12))
        mkp = ctx.enter_context(tc.tile_pool(name="mk", bufs=6))
        ptp = ctx.enter_context(tc.tile_pool(name="pt", bufs=3))
        ps_s = ctx.enter_context(tc.tile_pool(name="ps_s", bufs=2, space="PSUM"))
        ps_e = ctx.enter_context(tc.tile_pool(name="ps_e", bufs=2, space="PSUM"))
        ps_acc = ctx.enter_context(tc.tile_pool(name="ps_acc", bufs=1, space="PSUM"))

        def load_w(ext, kchunks, width, dtype=BF):
            # one DMA per k-chunk: spreads the transfer across DMA queues so a
            # single weight matrix never serializes ~1MB behind one queue
            t = wpool.tile([128, kchunks, width], dtype, tag=ext.name)
            ap = ext.ap().rearrange("(k p) d -> p k d", p=128)
            for k in range(kchunks):
                nc.sync.dma_start(t[:, k, :], ap[:, k, :])
            return t

        # phase A weights first (they gate the whole pipeline)
        gin_sb = [load_w(gin_e[a], 5, G) for a in range(2)]
        fc_sb = [load_w(fc_e[a], 5, HID) for a in range(2)]
        qw_sb = [load_w(qw_e[a], 7, HID) for a in range(2)]
        kw_sb = [load_w(kw_e[a], 6, HID) for a in range(2)]

        ones_col = wpool.tile([128, 1], BF)
        nc.vector.memset(ones_col[:], 1.0)
        zbias = wpool.tile([128, 1], F32)
        nc.vector.memset(zbias[:], 0.0)

        AFT = mybir.ActivationFunctionType

        # ---------------- phase A: q_tilde^T [768, 2G] ----------------
        qt_all = apool.tile([128, 6, 2 * G], BF)
        for a in range(2):
            # gT_aug [896-ish rows as [128, 7, G]]: chunks 0..5 = relu(fc @ geneT),
            # chunk 6 = ones row (row 768) + zeros.
            gt = apool.tile([128, 7, G], BF, tag=f"gt{a}")
            nc.vector.memset(gt[:, 6, :], 0.0)
            nc.vector.memset(gt[0:1, 6, :], 1.0)
            ps = ps_s.tile([128, 6, G], F32, tag="ps_a")
            for m in range(6):
                for k in range(5):
                    nc.tensor.matmul(
                        ps[:, m, :],
                        fc_sb[a][:, k, m * 128:(m + 1) * 128],
                        gin_sb[a][:, k, :],
                        start=(m == 0 and k == 0), stop=(m == 5 and k == 4),
                    )
            nc.scalar.activation(gt[:, 0:6, :], ps[:], AFT.Relu, bias=zbias[:])

            qtmp = apool.tile([128, 6, G], BF, tag=f"qtmp{a}")
            ps = ps_s.tile([128, 6, G], F32, tag="ps_a")
            for m in range(6):
                for k in range(7):
                    nc.tensor.matmul(
                        ps[:, m, :],
                        qw_sb[a][:, k, m * 128:(m + 1) * 128],
                        gt[:, k, :],
                        start=(m == 0 and k == 0), stop=(m == 5 and k == 6),
                    )
            nc.vector.tensor_copy(qtmp[:], ps[:])

            ps = ps_s.tile([128, 6, G], F32, tag="ps_a")
            for m in range(6):
                for k in range(6):
                    nc.tensor.matmul(
                        ps[:, m, :],
                        kw_sb[a][:, k, m * 128:(m + 1) * 128],
                        qtmp[:, k, :],
                        start=(m == 0 and k == 0), stop=(m == 5 and k == 5),
                    )
            nc.vector.tensor_copy(qt_all[:, :, a * G:(a + 1) * G], ps[:])

        if dbg:
            qtf = apool.tile([128, 6, 2 * G], F32, tag="dbg_qtf")
            nc.vector.tensor_copy(qtf[:], qt_all[:])
            nc.sync.dma_start(dbg_e["qt_all"].ap(), qtf[:])

        # ---------------- main loop: energies, exp, mask, context ----------------
        ctx_ps = ps_acc.tile([128, 6, 2 * G], F32)   # ctx^T chunks, accumulated
        l_ps = ps_acc.tile([1, 4, 2 * G], F32)       # per-j partial softmax denominators

        # phase C weights are issued mid-loop: they are needed only at the end,
        # and issuing them late keeps the first X supertiles at the head of the
        # DMA queues (v1 showed a 14us PE stall waiting for the first xt tile).
        vw_sb = [None, None]
        ow_sb = [None, None]
        ob_sb = wpool.tile([128, 6], F32)
        for t in range(ns):
            if t == min(3, ns - 1):
                for a in range(2):
                    vw_sb[a] = load_w(vw_e[a], 6, HID)
                    ow_sb[a] = load_w(ow_e[a], 6, HID)
                nc.sync.dma_start(ob_sb[:], ob_e.ap())
            xt_t = xtp.tile([128, 6, T], BF)
            nc.sync.dma_start(xt_t[:], xt_e.ap()[t])
            mk_t = mkp.tile([128, 4, 2 * G], BF)
            nc.sync.dma_start(mk_t[:], mk_e.ap()[t])

            xn_ts = []
            for j in range(4):
                xn_t = xnp.tile([128, HID], BF)
                nc.sync.dma_start(xn_t[:], xn_e.ap()[t * T + j * 128: t * T + (j + 1) * 128, :])
                xn_ts.append(xn_t)
            et = ps_e.tile([128, 4, 2 * G], F32)
            for j in range(4):
                for h in range(6):
                    nc.tensor.matmul(
                        et[:, j, :],
                        xt_t[:, h, j * 128:(j + 1) * 128],
                        qt_all[:, h, :],
                        start=(j == 0 and h == 0), stop=(j == 3 and h == 5),
                    )
            pexp = ptp.tile([128, 4, 2 * G], BF, tag="pexp")
            nc.scalar.activation(pexp[:], et[:], AFT.Exp, bias=zbias[:], scale=SCALE)
            pt = ptp.tile([128, 4, 2 * G], BF, tag="pt")
            nc.vector.tensor_mul(pt[:], pexp[:], mk_t[:])
            if dbg and t == 0:
                ptf = apool.tile([128, 4, 2 * G], F32, tag="dbg_ptf")
                nc.vector.tensor_copy(ptf[:], pt[:])
                nc.sync.dma_start(dbg_e["pt0"].ap(), ptf[:])

            for j in range(4):
                xn_t = xn_ts[j]
                for h in range(6):
                    nc.tensor.matmul(
                        ctx_ps[:, h, :],
                        xn_t[:, h * 128:(h + 1) * 128],
                        pt[:, j, :],
                        start=(t == 0 and j == 0 and h == 0),
                        stop=(t == ns - 1 and j == 3 and h == 5),
                    )
            nc.tensor.matmul(
                l_ps[:], ones_col[:], pt[:],
                start=(t == 0), stop=(t == ns - 1),
            )

        # ---------------- softmax denominators -> normalized ctx^T ----------------
        l4_sb = apool.tile([1, 4, 2 * G], F32)
        nc.vector.tensor_copy(l4_sb[:], l_ps[:])
        l_sb = apool.tile([1, 2 * G], F32)
        ltmp = apool.tile([1, 2 * G], F32)
        nc.vector.tensor_add(l_sb[:], l4_sb[:, 0, :], l4_sb[:, 1, :])
        nc.vector.tensor_add(ltmp[:], l4_sb[:, 2, :], l4_sb[:, 3, :])
        nc.vector.tensor_add(l_sb[:], l_sb[:], ltmp[:])
        rinv = apool.tile([1, 2 * G], F32)
        nc.vector.reciprocal(rinv[:], l_sb[:])
        rrep = apool.tile([128, 2 * G], F32)
        nc.gpsimd.partition_broadcast(rrep[:], rinv[:])

        ctxT = apool.tile([128, 6, 2 * G], BF)
        for h in range(6):
            nc.vector.tensor_mul(ctxT[:, h, :], ctx_ps[:, h, :], rrep[:])

        if dbg:
            ctxf = apool.tile([128, 6, 2 * G], F32, tag="dbg_ctxf")
            nc.vector.tensor_copy(ctxf[:], ctx_ps[:])
            nc.sync.dma_start(dbg_e["ctx"].ap(), ctxf[:])
            nc.sync.dma_start(dbg_e["l"].ap(), l4_sb[:])
            ctxtf = apool.tile([128, 6, 2 * G], F32, tag="dbg_ctxtf")
            nc.vector.tensor_copy(ctxtf[:], ctxT[:])
            nc.sync.dma_start(dbg_e["ctxT"].ap(), ctxtf[:])

        # ---------------- phase C: out^T = sum_a ow_a (vw_a ctx_a^T) + bias ----------------
        hts = []
        for a in range(2):
            ht = apool.tile([128, 6, G], BF, tag=f"ht{a}")
            ps = ps_s.tile([128, 6, G], F32, tag="ps_c")
            for m in range(6):
                for k in range(6):
                    nc.tensor.matmul(
                        ps[:, m, :],
                        vw_sb[a][:, k, m * 128:(m + 1) * 128],
                        ctxT[:, k, a * G:(a + 1) * G],
                        start=(m == 0 and k == 0), stop=(m == 5 and k == 5),
                    )
            nc.vector.tensor_copy(ht[:], ps[:])
            hts.append(ht)

        o_sb = apool.tile([128, 6, G], F32)
        ps = ps_s.tile([128, 6, G], F32, tag="ps_c")
        for m in range(6):
            for a in range(2):
                for k in range(6):
                    nc.tensor.matmul(
                        ps[:, m, :],
                        ow_sb[a][:, k, m * 128:(m + 1) * 128],
                        hts[a][:, k, :],
                        start=(m == 0 and a == 0 and k == 0),
                        stop=(m == 5 and a == 1 and k == 5),
                    )
        for m in range(6):
            nc.scalar.activation(
                o_sb[:, m, :], ps[:, m, :], AFT.Identity,
                bias=ob_sb[:, m:m + 1], scale=1.0,
            )
        nc.sync.dma_start(out_e.ap().rearrange("(m p) i -> p m i", p=128), o_sb[:])

    nc.compile()
    return nc


def _prep_inputs(x, batch_ids, gene, bionic, p):
    """Shard + lay out all per-core numpy inputs. Returns (in_maps, C)."""
    bids = np.asarray(batch_ids).astype(np.int64)
    x = np.asarray(x, dtype=np.float32)
    gene = np.asarray(gene, dtype=np.float32)
    bionic = np.asarray(bionic, dtype=np.float32)

    bounds = np.searchsorted(bids, np.arange(0, B + 1, G))
    counts = np.diff(bounds)
    C = int(np.ceil(max(int(counts.max()), 1) / float(T)) * T)
    ns = C // T

    # replicated params (host-side transposes / bias folds only)
    def bf(a):
        return np.ascontiguousarray(a).astype(BF16)

    fcT = []
    for w, b_ in ((p["fc0_w"], p["fc0_b"]), (p["fc1_w"], p["fc1_b"])):
        t = np.zeros((640, HID), np.float32)
        t[:GENE] = np.asarray(w, np.float32).T
        t[GENE] = np.asarray(b_, np.float32)
        fcT.append(bf(t))
    qwT = []
    for w, b_ in ((p["a0_qw"], p["a0_qb"]), (p["a1_qw"], p["a1_qb"])):
        t = np.zeros((896, HID), np.float32)
        t[:HID] = np.asarray(w, np.float32).T
        t[HID] = np.asarray(b_, np.float32)
        qwT.append(bf(t))
    kwN = [bf(np.asarray(p["a0_kw"], np.float32)), bf(np.asarray(p["a1_kw"], np.float32))]
    vwT = [bf(np.asarray(p["a0_vw"], np.float32).T), bf(np.asarray(p["a1_vw"], np.float32).T)]
    owT = [bf(np.asarray(p["a0_ow"], np.float32).T), bf(np.asarray(p["a1_ow"], np.float32).T)]
    out_bias = (
        np.asarray(p["a0_vb"], np.float32) @ np.asarray(p["a0_ow"], np.float32).T
        + np.asarray(p["a0_ob"], np.float32)
        + np.asarray(p["a1_vb"], np.float32) @ np.asarray(p["a1_ow"], np.float32).T
        + np.asarray(p["a1_ob"], np.float32)
    )
    ob_pb = np.ascontiguousarray(out_bias.reshape(6, 128).T).astype(np.float32)

    in_maps = []
    for c in range(NCORES):
        s, e = int(bounds[c]), int(bounds[c + 1])
        cnt = e - s
        xs = np.zeros((C, HID), np.float32)
        xs[:cnt] = x[s:e]
        xb = xs.astype(BF16)
        xt4 = np.ascontiguousarray(
            xb.T.reshape(6, 128, ns, T).transpose(2, 1, 0, 3)
        )
        lab = np.full((C,), -1, np.int64)
        lab[:cnt] = bids[s:e] - c * G
        m16 = (lab[:, None] == np.arange(G)[None, :])
        m32 = np.concatenate([m16, m16], axis=1).astype(BF16)
        mask4 = np.ascontiguousarray(m32.reshape(ns, 4, 128, 2 * G).transpose(0, 2, 1, 3))

        def gT(v):
            t = np.zeros((640, G), np.float32)
            t[:GENE] = v[c * G:(c + 1) * G].T
            t[GENE] = 1.0
            return t.astype(BF16)

        in_maps.append({
            "xt4": xt4,
            "xn": xb,
            "mask4": mask4,
            "geneT": gT(gene),
            "bionT": gT(bionic),
            "fc0T": fcT[0], "fc1T": fcT[1],
            "qw0T": qwT[0], "qw1T": qwT[1],
            "kw0": kwN[0], "kw1": kwN[1],
            "vw0T": vwT[0], "vw1T": vwT[1],
            "ow0T": owT[0], "ow1T": owT[1],
            "out_bias_pb": ob_pb,
        })
    return in_maps, C


def kernel(**inputs):
    x = inputs["x"]
    batch_ids = inputs["batch_ids"]
    gene = inputs["gene"]
    bionic = inputs["bionic"]
    in_maps, C = _prep_inputs(x, batch_ids, gene, bionic, inputs)

    if C not in _BUILD_CACHE:
        _BUILD_CACHE[C] = _build(C)
    nc = _BUILD_CACHE[C]

    prof_dir = os.environ.get("BASSK_PROFILE_DIR")
    if prof_dir:
        from trn_agent_boot.trn_boot import _ntff_profile_via_ctypes
        hook = _ntff_profile_via_ctypes("/opt/axon/libaxon_pjrt.so")
        os.makedirs(prof_dir, exist_ok=True)
        with hook(prof_dir, [0]):
            res = run_bass_kernel_spmd(nc, in_maps, core_ids=list(range(NCORES)))
        kernel.last_nc = nc
    else:
        res = run_bass_kernel_spmd(nc, in_maps, core_ids=list(range(NCORES)))

    out = np.empty((B, HID), np.float32)
    for c in range(NCORES):
        out[c * G:(c + 1) * G, :] = res.results[c]["out"].T
    return out



# revision 6
# speedup vs baseline: 1.5701x; 1.5701x over previous
"""Ragged-batch dual single-head attention (AttentionLayer) for Trainium2, 8 NeuronCores.

Data-parallel over graphs: 16 graphs per core, contiguous node segments
(batch_ids is sorted). The device only does the O(N) part; everything that is
O(B) with heavy weight traffic runs on the host:

  host:   g = relu(fc gene), Q = g qw^T + qb, q_tilde = Q kw   (per graph, tiny)
  device: e[n,g] = x[n].q_tilde[g]/sqrt(768);  p = exp(e)*mask
          ctx     = p^T X   (unnormalized), l[g] = sum_n p[n,g]
  host:   out = (ctx/l) @ (ow vw)^T + (vb ow^T + ob)           (per graph, tiny)

This removes all replicated 768x768 weight DMA (~11.8MB/core) from the device.

The graph mask is folded into the energy matmul: a 16-row one-hot block
(2.0 * one-hot of the node's local graph id) contracts against a [16, 2G]
matrix holding 224 on the own-graph columns (both exact in fp8e4m3, max 224);
the +448 own-graph product is cancelled by the exp bias, so wrong-graph /
padded entries get weight exp(-448/sqrt(768)) ~ 1e-7 ~ 0.

Energies run as fp8 DoubleRow matmuls (K=256 per instruction). The context
matmul uses the attention weights as the stationary operand and streams
x (fp16) 772 columns wide; column 768 is ones, so the softmax denominator
accumulates as context column 768. f32 PSUM throughout.
"""

import os
from contextlib import ExitStack

import numpy as np
import ml_dtypes

import concourse.bass as bass
import concourse.tile as tile
from concourse import bacc, mybir
from concourse.bass_utils import run_bass_kernel_spmd

BF16 = ml_dtypes.bfloat16
FP16 = np.float16
FP8 = ml_dtypes.float8_e4m3
HID = 768
XW = 772          # xn columns: 768 x + 1 ones (softmax denom) + 3 zero pad
GENE = 512
B = 128
NCORES = 8
G = B // NCORES   # graphs per core
TG = 2 * G        # two attentions' worth of graph columns
T = 512           # nodes per supertile
SCALE = 1.0 / float(np.sqrt(HID))
LARGE = 448.0
OH_VAL = 2.0
QTOH_VAL = LARGE / OH_VAL

_BUILD_CACHE = {}


def _build(C, num_devices=NCORES):
    """Build + compile the SPMD Bass graph for per-core node capacity C."""
    ns = C // T
    dt = mybir.dt
    F16 = dt.float16
    F32 = dt.float32
    F8 = dt.float8e4
    DR = mybir.MatmulPerfMode.DoubleRow

    nc = bacc.Bacc("TRN2", target_bir_lowering=False, debug=False, num_devices=num_devices)

    xt_e = nc.declare_dram_parameter("xt4", [ns, 128, 3, 2, T], F8, isOutput=False)
    oh_e = nc.declare_dram_parameter("oh4", [ns, 16, T], F8, isOutput=False)
    xn_e = nc.declare_dram_parameter("xn", [ns, 4, 128, XW], F16, isOutput=False)
    qt_e = nc.declare_dram_parameter("qt_pb", [128, 3, 2, TG], F8, isOutput=False)
    qo_e = nc.declare_dram_parameter("qtoh", [16, TG], F8, isOutput=False)
    ctx_e = nc.declare_dram_parameter("ctxo", [TG, XW], F32, isOutput=True)

    with tile.TileContext(nc) as tc, ExitStack() as ctx:
        wpool = ctx.enter_context(tc.tile_pool(name="w", bufs=1))
        apool = ctx.enter_context(tc.tile_pool(name="fin", bufs=1))
        xtp = ctx.enter_context(tc.tile_pool(name="xt", bufs=4))
        ohp = ctx.enter_context(tc.tile_pool(name="oh", bufs=4))
        xnp = ctx.enter_context(tc.tile_pool(name="xn", bufs=4))
        ptp = ctx.enter_context(tc.tile_pool(name="pt", bufs=3))
        ps_e = ctx.enter_context(tc.tile_pool(name="ps_e", bufs=2, space="PSUM"))
        ps_acc = ctx.enter_context(tc.tile_pool(name="ps_acc", bufs=1, space="PSUM"))

        qt_sb = wpool.tile([128, 3, 2, TG], F8)
        nc.sync.dma_start(qt_sb[:], qt_e.ap())
        qo_sb = wpool.tile([16, TG], F8)
        nc.sync.dma_start(qo_sb[:], qo_e.ap())
        ebias = wpool.tile([128, 1], F32)
        nc.vector.memset(ebias[:], -LARGE * SCALE)

        AFT = mybir.ActivationFunctionType

        ctx_a = ps_acc.tile([TG, 512], F32)       # ctx cols 0..511
        ctx_b = ps_acc.tile([TG, XW - 512], F32)  # ctx cols 512..771 (768 = denom)

        for t in range(ns):
            xt_t = xtp.tile([128, 3, 2, T], F8)
            nc.sync.dma_start(xt_t[:], xt_e.ap()[t])
            oh_t = ohp.tile([16, T], F8)
            nc.sync.dma_start(oh_t[:], oh_e.ap()[t])
            xn_t = xnp.tile([128, 4, XW], F16)
            nc.sync.dma_start(xn_t[:], xn_e.ap().rearrange("t j p d -> t p j d")[t])

            et = ps_e.tile([128, 4, TG], F32)
            for j in range(4):
                for hp in range(3):
                    nc.tensor.matmul(
                        et[:, j, :],
                        xt_t[:, hp, :, j * 128:(j + 1) * 128],
                        qt_sb[:, hp, :, :],
                        start=(j == 0 and hp == 0), stop=False,
                        perf_mode=DR,
                    )
                nc.tensor.matmul(
                    et[:, j, :],
                    oh_t[:, j * 128:(j + 1) * 128],
                    qo_sb[:],
                    start=False, stop=(j == 3),
                )
            pexp = ptp.tile([128, 4, TG], F16, tag="pexp")
            nc.scalar.activation(pexp[:], et[:], AFT.Exp, bias=ebias[:], scale=SCALE)

            for j in range(4):
                nc.tensor.matmul(
                    ctx_a[:], pexp[:, j, :], xn_t[:, j, 0:512],
                    start=(t == 0 and j == 0), stop=(t == ns - 1 and j == 3),
                )
                nc.tensor.matmul(
                    ctx_b[:], pexp[:, j, :], xn_t[:, j, 512:XW],
                    start=(t == 0 and j == 0), stop=(t == ns - 1 and j == 3),
                )

        ctx_sb = apool.tile([TG, XW], F32)
        nc.vector.tensor_copy(ctx_sb[:, 0:512], ctx_a[:])
        nc.vector.tensor_copy(ctx_sb[:, 512:XW], ctx_b[:])
        nc.sync.dma_start(ctx_e.ap(), ctx_sb[:])

    nc.compile()
    return nc


def _prep_inputs(x, batch_ids, gene, bionic, p):
    """Shard + lay out per-core numpy inputs; compute q_tilde on host."""
    bids = np.asarray(batch_ids).astype(np.int64)
    x = np.asarray(x, dtype=np.float32)
    gene = np.asarray(gene, dtype=np.float32)
    bionic = np.asarray(bionic, dtype=np.float32)

    bounds = np.searchsorted(bids, np.arange(0, B + 1, G))
    counts = np.diff(bounds)
    C = int(np.ceil(max(int(counts.max()), 1) / float(T)) * T)
    ns = C // T

    # ---- host phase A: q_tilde per graph, both attentions ----
    qts = []
    for feat, fw, fb, l in ((gene, p["fc0_w"], p["fc0_b"], "a0"),
                            (bionic, p["fc1_w"], p["fc1_b"], "a1")):
        gf = np.maximum(feat @ np.asarray(fw, np.float32).T + np.asarray(fb, np.float32), 0.0)
        Q = gf @ np.asarray(p[f"{l}_qw"], np.float32).T + np.asarray(p[f"{l}_qb"], np.float32)
        qts.append(Q @ np.asarray(p[f"{l}_kw"], np.float32))  # [B, HID] = q_tilde rows
    qt_all = np.stack(qts, axis=0)  # [2, B, HID]

    qtoh = np.zeros((16, TG), np.float32)
    for k in range(G):
        qtoh[k, k] = QTOH_VAL
        qtoh[k, k + G] = QTOH_VAL
    qtoh = qtoh.astype(FP8)

    in_maps = []
    for c in range(NCORES):
        s, e = int(bounds[c]), int(bounds[c + 1])
        cnt = e - s
        xs = np.zeros((C, XW), np.float32)
        xs[:cnt, :HID] = x[s:e]
        xs[:, HID] = 1.0
        # DoubleRow-packed x^T: [t, p, hpair, i, n] = x^T[hpair*256 + i*128 + p, n]
        xt4 = np.ascontiguousarray(
            xs[:, :HID].astype(FP8).T.reshape(3, 2, 128, ns, T).transpose(3, 2, 0, 1, 4)
        )
        lab = np.full((C,), -1, np.int64)
        lab[:cnt] = bids[s:e] - c * G
        oh = OH_VAL * (lab[None, :] == np.arange(G)[:, None]).astype(np.float32)  # [16, C]
        oh4 = np.ascontiguousarray(oh.reshape(G, ns, T).transpose(1, 0, 2)).astype(FP8)

        # q_tilde columns for this core's graphs, DoubleRow packed
        qt = np.concatenate([qt_all[0, c * G:(c + 1) * G].T,
                             qt_all[1, c * G:(c + 1) * G].T], axis=1)  # [768, 2G]
        qt_pb = np.ascontiguousarray(qt.reshape(3, 2, 128, TG).transpose(2, 0, 1, 3)).astype(FP8)

        in_maps.append({
            "xt4": xt4,
            "oh4": oh4,
            "xn": np.ascontiguousarray(xs.astype(FP16).reshape(ns, 4, 128, XW)),
            "qt_pb": qt_pb,
            "qtoh": qtoh,
        })
    return in_maps, C


def kernel(**inputs):
    x = inputs["x"]
    batch_ids = inputs["batch_ids"]
    gene = inputs["gene"]
    bionic = inputs["bionic"]
    in_maps, C = _prep_inputs(x, batch_ids, gene, bionic, inputs)

    if C not in _BUILD_CACHE:
        _BUILD_CACHE[C] = _build(C)
    nc = _BUILD_CACHE[C]

    prof_dir = os.environ.get("BASSK_PROFILE_DIR")
    if prof_dir:
        from trn_agent_boot.trn_boot import _ntff_profile_via_ctypes
        hook = _ntff_profile_via_ctypes("/opt/axon/libaxon_pjrt.so")
        os.makedirs(prof_dir, exist_ok=True)
        with hook(prof_dir, [0]):
            res = run_bass_kernel_spmd(nc, in_maps, core_ids=list(range(NCORES)))
        kernel.last_nc = nc
    else:
        res = run_bass_kernel_spmd(nc, in_maps, core_ids=list(range(NCORES)))

    # ---- host phase C: normalize and project ----
    p32 = lambda k: np.asarray(inputs[k], np.float32)
    wvo0 = p32("a0_ow") @ p32("a0_vw")
    wvo1 = p32("a1_ow") @ p32("a1_vw")
    out_bias = (p32("a0_vb") @ p32("a0_ow").T + p32("a0_ob")
                + p32("a1_vb") @ p32("a1_ow").T + p32("a1_ob"))

    out = np.empty((B, HID), np.float32)
    for c in range(NCORES):
        ctxo = res.results[c]["ctxo"]             # [2G, XW]
        ctxn = ctxo[:, :HID] / ctxo[:, HID][:, None]
        out[c * G:(c + 1) * G] = (ctxn[:G] @ wvo0.T + ctxn[G:] @ wvo1.T + out_bias)
    return out


# revision 11
# speedup vs baseline: 2.0041x; 1.2764x over previous
"""Ragged-batch dual single-head attention (AttentionLayer) for Trainium2, 8 NeuronCores.

Data-parallel over graphs: 16 graphs per core, contiguous node segments
(batch_ids is sorted). The device only does the O(N) part; everything that is
O(B) with heavy weight traffic runs on the host:

  host:   g = relu(fc gene), Q = g qw^T + qb, q_tilde = Q kw   (per graph, tiny)
  device: e[n,g] = x[n].q_tilde[g]/sqrt(768);  p = exp(e)*mask
          ctx     = p^T X   (unnormalized), l[g] = sum_n p[n,g]
  host:   out = (ctx/l) @ (ow vw)^T + (vb ow^T + ob)           (per graph, tiny)

This removes all replicated 768x768 weight DMA (~11.8MB/core) from the device.

The graph mask is folded into the energy matmul: a 16-row one-hot block
(2.0 * one-hot of the node's local graph id) contracts against a [16, 2G]
matrix holding 224 on the own-graph columns (both exact in fp8e4m3, max 224);
the +448 own-graph product is cancelled by the exp bias, so wrong-graph /
padded entries get weight exp(-448/sqrt(768)) ~ 1e-7 ~ 0.

Energies run as fp8 DoubleRow matmuls (K=256 per instruction). The context
matmul uses the attention weights as the stationary operand and streams
x (fp16) 772 columns wide; column 768 is ones, so the softmax denominator
accumulates as context column 768. f32 PSUM throughout.
"""

import os
from contextlib import ExitStack

import numpy as np
import ml_dtypes

import concourse.bass as bass
import concourse.tile as tile
from concourse import bacc, mybir
from concourse.bass_utils import run_bass_kernel_spmd

BF16 = ml_dtypes.bfloat16
FP16 = np.float16
FP8 = ml_dtypes.float8_e4m3
HID = 768
XW = 772          # xn columns: 768 x + 1 ones (softmax denom) + 3 zero pad
GENE = 512
B = 128
NCORES = 8
G = B // NCORES   # graphs per core
TG = 2 * G        # two attentions' worth of graph columns
T = 512           # nodes per supertile
SCALE = 1.0 / float(np.sqrt(HID))
LARGE = 448.0
OH_VAL = 2.0
QTOH_VAL = LARGE / OH_VAL

_BUILD_CACHE = {}


def _build(C, num_devices=NCORES):
    """Build + compile the SPMD Bass graph for per-core node capacity C."""
    ns = C // T
    dt = mybir.dt
    F16 = dt.float16
    F32 = dt.float32
    F8 = dt.float8e4
    DR = mybir.MatmulPerfMode.DoubleRow

    nc = bacc.Bacc("TRN2", target_bir_lowering=False, debug=False, num_devices=num_devices)

    xt_e = nc.declare_dram_parameter("xt4", [ns, 128, 3, 2, T], F8, isOutput=False)
    oh_e = nc.declare_dram_parameter("oh4", [ns, 16, T], F8, isOutput=False)
    xn_e = nc.declare_dram_parameter("xn", [ns, 4, 128, HID], F16, isOutput=False)
    qt_e = nc.declare_dram_parameter("qt_pb", [128, 3, 2, TG], F8, isOutput=False)
    qo_e = nc.declare_dram_parameter("qtoh", [16, TG], F8, isOutput=False)
    ctx_e = nc.declare_dram_parameter("ctx4", [128, 6, TG], F32, isOutput=True)
    l_e = nc.declare_dram_parameter("l4", [1, 4, TG], F32, isOutput=True)

    with tile.TileContext(nc) as tc, ExitStack() as ctx:
        wpool = ctx.enter_context(tc.tile_pool(name="w", bufs=1))
        apool = ctx.enter_context(tc.tile_pool(name="fin", bufs=1))
        xtp = ctx.enter_context(tc.tile_pool(name="xt", bufs=4))
        ohp = ctx.enter_context(tc.tile_pool(name="oh", bufs=4))
        xnp = ctx.enter_context(tc.tile_pool(name="xn", bufs=4))
        ptp = ctx.enter_context(tc.tile_pool(name="pt", bufs=3))
        ps_e = ctx.enter_context(tc.tile_pool(name="ps_e", bufs=2, space="PSUM"))
        ps_acc = ctx.enter_context(tc.tile_pool(name="ps_acc", bufs=1, space="PSUM"))

        qt_sb = wpool.tile([128, 3, 2, TG], F8)
        nc.sync.dma_start(qt_sb[:], qt_e.ap())
        qo_sb = wpool.tile([16, TG], F8)
        nc.sync.dma_start(qo_sb[:], qo_e.ap())
        ones_col = wpool.tile([128, 1], F16)
        nc.vector.memset(ones_col[:], 1.0)
        ebias = wpool.tile([128, 1], F32)
        nc.vector.memset(ebias[:], -LARGE * SCALE)

        AFT = mybir.ActivationFunctionType

        ctx_ps = ps_acc.tile([128, 6, TG], F32)   # ctx^T chunks, accumulated
        l_ps = ps_acc.tile([1, 4, TG], F32)       # per-j partial softmax denominators

        for t in range(ns):
            xt_t = xtp.tile([128, 3, 2, T], F8)
            nc.sync.dma_start(xt_t[:], xt_e.ap()[t])
            oh_t = ohp.tile([16, T], F8)
            nc.sync.dma_start(oh_t[:], oh_e.ap()[t])
            xn_t = xnp.tile([128, 4, HID], F16)
            nc.sync.dma_start(xn_t[:], xn_e.ap().rearrange("t j p d -> t p j d")[t])

            et = ps_e.tile([128, 4, TG], F32)
            for j in range(4):
                for hp in range(3):
                    nc.tensor.matmul(
                        et[:, j, :],
                        xt_t[:, hp, :, j * 128:(j + 1) * 128],
                        qt_sb[:, hp, :, :],
                        start=(j == 0 and hp == 0), stop=False,
                        perf_mode=DR,
                    )
                nc.tensor.matmul(
                    et[:, j, :],
                    oh_t[:, j * 128:(j + 1) * 128],
                    qo_sb[:],
                    start=False, stop=(j == 3),
                )
            pexp = ptp.tile([128, 4, TG], F16, tag="pexp")
            nc.scalar.activation(pexp[:], et[:], AFT.Exp, bias=ebias[:], scale=SCALE)

            for j in range(4):
                for m in range(6):
                    nc.tensor.matmul(
                        ctx_ps[:, m, :],
                        xn_t[:, j, m * 128:(m + 1) * 128],
                        pexp[:, j, :],
                        start=(t == 0 and j == 0 and m == 0),
                        stop=(t == ns - 1 and j == 3 and m == 5),
                    )
            nc.tensor.matmul(
                l_ps[:], ones_col[:], pexp[:],
                start=(t == 0), stop=(t == ns - 1),
            )

        ctx_sb = apool.tile([128, 6, TG], F32)
        nc.vector.tensor_copy(ctx_sb[:], ctx_ps[:])
        nc.sync.dma_start(ctx_e.ap(), ctx_sb[:])
        l_sb = apool.tile([1, 4, TG], F32)
        nc.vector.tensor_copy(l_sb[:], l_ps[:])
        nc.sync.dma_start(l_e.ap(), l_sb[:])

    nc.compile()
    return nc


def _prep_inputs(x, batch_ids, gene, bionic, p):
    """Shard + lay out per-core numpy inputs; compute q_tilde on host."""
    bids = np.asarray(batch_ids).astype(np.int64)
    x = np.asarray(x, dtype=np.float32)
    gene = np.asarray(gene, dtype=np.float32)
    bionic = np.asarray(bionic, dtype=np.float32)

    bounds = np.searchsorted(bids, np.arange(0, B + 1, G))
    counts = np.diff(bounds)
    C = int(np.ceil(max(int(counts.max()), 1) / float(T)) * T)
    ns = C // T

    # ---- host phase A: q_tilde per graph, both attentions ----
    qts = []
    for feat, fw, fb, l in ((gene, p["fc0_w"], p["fc0_b"], "a0"),
                            (bionic, p["fc1_w"], p["fc1_b"], "a1")):
        gf = np.maximum(feat @ np.asarray(fw, np.float32).T + np.asarray(fb, np.float32), 0.0)
        Q = gf @ np.asarray(p[f"{l}_qw"], np.float32).T + np.asarray(p[f"{l}_qb"], np.float32)
        qts.append(Q @ np.asarray(p[f"{l}_kw"], np.float32))  # [B, HID] = q_tilde rows
    qt_all = np.stack(qts, axis=0)  # [2, B, HID]

    qtoh = np.zeros((16, TG), np.float32)
    for k in range(G):
        qtoh[k, k] = QTOH_VAL
        qtoh[k, k + G] = QTOH_VAL
    qtoh = qtoh.astype(FP8)

    in_maps = []
    for c in range(NCORES):
        s, e = int(bounds[c]), int(bounds[c + 1])
        cnt = e - s
        xs = np.zeros((C, HID), np.float32)
        xs[:cnt] = x[s:e]
        # DoubleRow-packed x^T: [t, p, hpair, i, n] = x^T[hpair*256 + i*128 + p, n]
        xt4 = np.ascontiguousarray(
            xs.astype(FP8).T.reshape(3, 2, 128, ns, T).transpose(3, 2, 0, 1, 4)
        )
        lab = np.full((C,), -1, np.int64)
        lab[:cnt] = bids[s:e] - c * G
        oh = OH_VAL * (lab[None, :] == np.arange(G)[:, None]).astype(np.float32)  # [16, C]
        oh4 = np.ascontiguousarray(oh.reshape(G, ns, T).transpose(1, 0, 2)).astype(FP8)

        # q_tilde columns for this core's graphs, DoubleRow packed
        qt = np.concatenate([qt_all[0, c * G:(c + 1) * G].T,
                             qt_all[1, c * G:(c + 1) * G].T], axis=1)  # [768, 2G]
        qt_pb = np.ascontiguousarray(qt.reshape(3, 2, 128, TG).transpose(2, 0, 1, 3)).astype(FP8)

        in_maps.append({
            "xt4": xt4,
            "oh4": oh4,
            "xn": np.ascontiguousarray(xs.astype(FP16).reshape(ns, 4, 128, HID)),
            "qt_pb": qt_pb,
            "qtoh": qtoh,
        })
    return in_maps, C


def kernel(**inputs):
    x = inputs["x"]
    batch_ids = inputs["batch_ids"]
    gene = inputs["gene"]
    bionic = inputs["bionic"]
    in_maps, C = _prep_inputs(x, batch_ids, gene, bionic, inputs)

    if C not in _BUILD_CACHE:
        _BUILD_CACHE[C] = _build(C)
    nc = _BUILD_CACHE[C]

    prof_dir = os.environ.get("BASSK_PROFILE_DIR")
    if prof_dir:
        from trn_agent_boot.trn_boot import _ntff_profile_via_ctypes
        hook = _ntff_profile_via_ctypes("/opt/axon/libaxon_pjrt.so")
        os.makedirs(prof_dir, exist_ok=True)
        with hook(prof_dir, [0]):
            res = run_bass_kernel_spmd(nc, in_maps, core_ids=list(range(NCORES)))
        kernel.last_nc = nc
    else:
        res = run_bass_kernel_spmd(nc, in_maps, core_ids=list(range(NCORES)))

    # ---- host phase C: normalize and project ----
    p32 = lambda k: np.asarray(inputs[k], np.float32)
    wvo0 = p32("a0_ow") @ p32("a0_vw")
    wvo1 = p32("a1_ow") @ p32("a1_vw")
    out_bias = (p32("a0_vb") @ p32("a0_ow").T + p32("a0_ob")
                + p32("a1_vb") @ p32("a1_ow").T + p32("a1_ob"))

    out = np.empty((B, HID), np.float32)
    for c in range(NCORES):
        ctxT = res.results[c]["ctx4"].transpose(1, 0, 2).reshape(HID, TG)
        l = res.results[c]["l4"].reshape(4, TG).sum(axis=0)
        ctxn = (ctxT / l[None, :]).T              # [2G, HID]
        out[c * G:(c + 1) * G] = (ctxn[:G] @ wvo0.T + ctxn[G:] @ wvo1.T + out_bias)
    return out
